# revision 33
# baseline (speedup 1.0000x reference)
"""Trainium2 Bass kernel for a 6-layer transformer encoder.

Problem: B=4, S=512, D=1024, F=4096, H=16 heads (depth 64), L=6 layers, fp32.

Sharding: batch x sequence. Core c handles batch element c//2 and token half
c%2 (256 tokens). Per layer the pair (2b, 2b+1) exchanges K/V halves (bf16)
with one 2-core AllGather through DRAM bounce buffers; the Q projection is
placed after the collective issue so PE keeps working during the exchange.

All matmul operands are bf16 (backend rejects mixed bf16 x f32r inputs, and
bf16 weights halve HBM traffic at the same PE speed). The residual stream is
f32 (hf/h1f) with bf16 twins (hb/h1b) written by the LN epilogue. LN stats
(f32r x f32r matmuls vs a ones column) are interleaved into the producing
loops (Wo / FFN) so the reduction is ready when the producers finish.
Attention: exp without max-subtraction, one exp per (t, kt) covering both
heads [128, 2x256]; row-sums ride along as a ones-column of V; the softmax
normalize is pipelined per d-tile under the next tile's matmuls. Elementwise
work is split between Scalar (exp, PSUM drains, bias adds) and Vector
(residuals, LN, relu, normalize) so neither becomes the bottleneck.
"""

import numpy as np

T = 512         # tokens per batch element (S)
TL = 256        # tokens per core (sequence split by 2)
D = 1024        # model dim
KD = D // 128   # 8 d-tiles
H = 16          # heads
DH = 64         # head dim
F = 4096        # ff dim
FT = F // 128   # 32 f-tiles
L = 6           # layers
EPS = 1e-6
MAX_POS = 1000
NCORES = 8
KCOLS = KD * TL       # 2048 K columns in the exchange payload
VCOLS = 2 * H * 65    # 2080 V columns (2 local key blocks, ones col at 64)
XCOLS = KCOLS + VCOLS # 4128

_cache = {}


def _imports():
    import sys
    try:
        import concourse.bass  # noqa
    except ImportError:
        for p in ("/opt/trn_rl_repo", "/root/.axon_site/_ro/trn_rl_repo"):
            if p not in sys.path:
                sys.path.insert(0, p)
    import concourse.bass as bass
    import concourse.mybir as mybir
    import concourse.tile as tile
    from concourse import bacc
    from concourse.bass_utils import run_bass_kernel_spmd
    return bass, mybir, tile, bacc, run_bass_kernel_spmd


def build(nlayers=L, debug=False):
    bass, mybir, tile, bacc, _ = _imports()
    f32 = mybir.dt.float32
    f32r = mybir.dt.float32r
    bf16 = mybir.dt.bfloat16
    AF = mybir.ActivationFunctionType
    OP = mybir.AluOpType

    nc = bacc.Bacc(None, target_bir_lowering=False, debug=True)

    # ---- kernel I/O ----
    xT = nc.declare_dram_parameter("xT", [D, TL], f32, isOutput=False)
    msk = nc.declare_dram_parameter("msk", [128, 4], f32, isOutput=False)
    Wq = nc.declare_dram_parameter("Wq", [L, D, D], bf16, isOutput=False)
    Wk = nc.declare_dram_parameter("Wk", [L, D, D], bf16, isOutput=False)
    Wv = nc.declare_dram_parameter("Wv", [L, D, D], bf16, isOutput=False)
    Wo = nc.declare_dram_parameter("Wo", [L, D, D], bf16, isOutput=False)
    W1 = nc.declare_dram_parameter("W1", [L, D, F], bf16, isOutput=False)
    W2 = nc.declare_dram_parameter("W2", [L, F, D], bf16, isOutput=False)
    bias9 = nc.declare_dram_parameter("bias9", [L, 128, KD, 9], f32, isOutput=False)
    b1h = nc.declare_dram_parameter("b1h", [L, 128, FT, 1], f32, isOutput=False)
    cst = nc.declare_dram_parameter("cst", [128, 65], f32r, isOutput=False)   # ones
    cstb = nc.declare_dram_parameter("cstb", [128, 32], bf16, isOutput=False)  # ones
    crow = nc.declare_dram_parameter("crow", [65, 128], f32r, isOutput=False)  # ones
    sel2 = nc.declare_dram_parameter("sel2", [2, 128], f32r, isOutput=False)
    out = nc.declare_dram_parameter("out", [D, TL], f32, isOutput=True)

    dbg = {}
    if debug:
        for name, shape, dt_ in [("do", [D, TL], bf16), ("dr1", [D, TL], f32),
                                 ("dh1", [D, TL], f32), ("du", [2048, TL], bf16),
                                 ("dr2", [D, TL], f32)]:
            dbg[name] = nc.declare_dram_parameter(name, shape, dt_, isOutput=True)

    def wrow(w):  # [D, D] -> [128, KD, D] view (k-partition tiles)
        return w.rearrange("(ko kp) m -> kp ko m", kp=128)

    RG = [[0, 1], [2, 3], [4, 5], [6, 7]]

    with tile.TileContext(nc) as tc:
        with tc.tile_pool(name="sb", bufs=1) as sb1, \
             tc.tile_pool(name="sb2", bufs=2) as sb2, \
             tc.tile_pool(name="sb3", bufs=3) as sb3, \
             tc.tile_pool(name="psA", bufs=2, space="PSUM") as psA, \
             tc.tile_pool(name="psB", bufs=2, space="PSUM") as psB, \
             tc.tile_pool(name="dram", bufs=2, space="DRAM") as dram:

            # ---- persistent tiles ----
            hf = sb1.tile([128, KD, TL], f32, tag="hf")    # residual stream
            hb = sb1.tile([128, KD, TL], bf16, tag="hb")   # matmul copy
            cst_sb = sb1.tile([128, 65], f32r, tag="cst")
            crow_sb = sb1.tile([65, 128], f32r, tag="crow")
            ks = sb1.tile([128, KD, TL], bf16, tag="ks")   # my K (t-strips)
            vs = sb1.tile([128, 2, H, 65], bf16, tag="vs") # my V (2 key blocks)
            oT = sb1.tile([128, KD, TL], f32, tag="oT")
            oTb = sb1.tile([128, KD, TL], bf16, tag="oTb")
            y2acc = sb1.tile([128, KD, TL], f32, tag="y2acc")
            msk_sb = sb1.tile([128, 4], f32, tag="msk")
            sel2_sb = sb1.tile([2, 128], f32r, tag="sel2")

            nc.sync.dma_start(hf[:], xT.rearrange("(ko kp) t -> kp ko t", kp=128))
            nc.sync.dma_start(cst_sb[:], cst[:])
            nc.sync.dma_start(crow_sb[:], crow[:])
            nc.sync.dma_start(msk_sb[:], msk[:])
            nc.sync.dma_start(sel2_sb[:], sel2[:])
            # ones column of vs (written once; data writes never touch col 64)
            with nc.allow_non_contiguous_dma(reason="tiny one-time ones-column fill"):
                nc.sync.dma_start(vs[:, :, :, 64], cstb[:])

            for o in range(KD):  # initial bf16 copy of the input
                nc.scalar.activation(hb[:, o, :], hf[:, o, :], AF.Copy)

            ones_col = cst_sb[:, 64:65]          # [128,1] f32r, stats lhsT
            onesr_ln = crow_sb[0:1, 0:128]       # [1,128] f32r @p0, LN bcast lhsT

            def ln_finish(st_s, st_q, r, dstb, dstf, g_col, be_col):
                """st_s/st_q: [1,TL] PSUM (sum, sumsq). r: [128,KD,TL] f32r."""
                negm = sb2.tile([1, TL], f32r, tag="negm", bufs=1)
                with nc.allow_low_precision(reason="LN stats rounding"):
                    nc.vector.tensor_scalar(negm[:], st_s[:], -1.0 / D, None, OP.mult)
                qs = sb2.tile([1, TL], f32, tag="lnscr", bufs=3)
                nc.vector.tensor_scalar(qs[:], st_q[:], 1.0 / D, EPS, OP.mult, OP.add)
                msq = sb2.tile([1, TL], f32, tag="lnscr", bufs=3)
                nc.vector.tensor_tensor(msq[:], negm[:].bitcast(f32), negm[:].bitcast(f32), OP.mult)
                var = sb2.tile([1, TL], f32, tag="lnscr", bufs=3)
                nc.vector.tensor_tensor(var[:], qs[:], msq[:], OP.subtract)
                vrec = sb2.tile([1, TL], f32, tag="lnscr", bufs=3)
                nc.vector.reciprocal_approx_fast(vrec[:], var[:])
                rstd = sb2.tile([1, TL], f32r, tag="rstd", bufs=1)
                with nc.allow_low_precision(reason="LN rstd rounding"):
                    nc.scalar.activation(rstd[:], vrec[:], AF.Sqrt)
                pnm = psB.tile([128, TL], f32, tag="aux")
                nc.tensor.matmul(pnm[:], onesr_ln, negm[:], start=True, stop=True)
                prs = psB.tile([128, TL], f32, tag="aux")
                nc.tensor.matmul(prs[:], onesr_ln, rstd[:], start=True, stop=True)
                for o in range(KD):
                    a = sb2.tile([128, TL], f32, tag="lna")
                    nc.vector.tensor_tensor(a[:], r[:, o, :].bitcast(f32), pnm[:], OP.add)
                    b = sb2.tile([128, TL], f32, tag="lnb")
                    nc.vector.tensor_tensor(b[:], a[:], prs[:], OP.mult)
                    nc.vector.tensor_scalar(dstf[:, o, :], b[:], g_col[:, o, :],
                                            be_col[:, o, :], OP.mult, OP.add)
                    nc.vector.tensor_scalar(dstb[:, o, :], b[:], g_col[:, o, :],
                                            be_col[:, o, :], OP.mult, OP.add)

            for l in range(nlayers):
                # ---- per-layer bias/gain staging (host-packed) ----
                bia = sb2.tile([128, KD, 9], f32, tag="bias")
                nc.sync.dma_start(bia[:], bias9[l])
                b1_sb = sb2.tile([128, FT, 1], f32, tag="b1")
                nc.sync.dma_start(b1_sb[:], b1h[l])

                # ===================== K projection =============================
                bnc_k = dram.tile([128, KD, TL], bf16, tag="bik")
                for tp in range(4):  # 2 d-tiles per weight chunk
                    wc = sb3.tile([128, KD, 256], bf16, tag="wsm", bufs=6)
                    nc.sync.dma_start(wc[:], wrow(Wk[l])[:, :, tp * 256:(tp + 1) * 256])
                    for ti in range(2):
                        t = 2 * tp + ti
                        pq = psA.tile([128, TL], f32, tag="ps", bufs=4)
                        for k in range(KD):
                            nc.tensor.matmul(pq[:], wc[:, k, ti * 128:(ti + 1) * 128],
                                             hb[:, k, :], start=(k == 0), stop=(k == KD - 1))
                        nc.scalar.activation(ks[:, t, :], pq[:], AF.Identity,
                                             bias=bia[:, t, 1:2])
                        nc.gpsimd.dma_start(bnc_k[:, t:t + 1, :], ks[:, t:t + 1, :])
                # K-gather: overlapped by the V and Q projections below
                bnc_ko = dram.tile([2, 128, KD, TL], bf16, tag="bok")
                nc.gpsimd.collective_compute(
                    "AllGather", OP.bypass, replica_groups=RG,
                    ins=[bnc_k.opt()], outs=[bnc_ko.opt()])
                kvg = sb2.tile([128, 2, XCOLS], bf16, tag="kvg", bufs=1)
                for g in range(2):
                    nc.gpsimd.dma_start(kvg[:, g, 0:KCOLS], bnc_ko[g])

                # ================= V projection (natural layout) ================
                # v[t, e] = sum_d h[d, t] * Wv[d, e]; lhsT = h chunk, rhs = Wv strip
                bnc_v = dram.tile([128, 2, H, 65], bf16, tag="biv")
                for nq in range(2):  # e-halves of 512 (one DMA, two matmul quarters)
                    wv_s = sb2.tile([128, KD, 512], bf16, tag="wmid", bufs=3)
                    nc.sync.dma_start(wv_s[:], wrow(Wv[l])[:, :, nq * 512:(nq + 1) * 512])
                    for nh in range(2):
                        for tt in range(2):  # local token blocks of 128
                            pv = psA.tile([128, 256], f32, tag="ps", bufs=4)
                            for k in range(KD):
                                nc.tensor.matmul(pv[:], hb[:, k, tt * 128:(tt + 1) * 128],
                                                 wv_s[:, k, nh * 256:(nh + 1) * 256],
                                                 start=(k == 0), stop=(k == KD - 1))
                            nc.scalar.activation(
                                vs[:, tt, nq * 8 + nh * 4:nq * 8 + nh * 4 + 4, 0:64],
                                pv[:], AF.Copy)
                    nc.gpsimd.dma_start(bnc_v[:, :, nq * 8:(nq + 1) * 8, :],
                                        vs[:, :, nq * 8:(nq + 1) * 8, :])
                # V-gather: overlapped by the Q projection and the logits phase
                bnc_vo = dram.tile([2, 128, 2, H, 65], bf16, tag="bov")
                nc.gpsimd.collective_compute(
                    "AllGather", OP.bypass, replica_groups=RG,
                    ins=[bnc_v.opt()], outs=[bnc_vo.opt()])
                for g in range(2):
                    nc.gpsimd.dma_start(kvg[:, g, KCOLS:XCOLS], bnc_vo[g])

                # ============ Q projection (overlaps the collectives) ===========
                qall = sb2.tile([128, KD, TL], bf16, tag="qall")
                for tp in range(4):
                    wc = sb3.tile([128, KD, 256], bf16, tag="wsm", bufs=6)
                    nc.sync.dma_start(wc[:], wrow(Wq[l])[:, :, tp * 256:(tp + 1) * 256])
                    for ti in range(2):
                        t = 2 * tp + ti
                        pq = psA.tile([128, TL], f32, tag="ps", bufs=4)
                        for k in range(KD):
                            nc.tensor.matmul(pq[:], wc[:, k, ti * 128:(ti + 1) * 128],
                                             hb[:, k, :], start=(k == 0), stop=(k == KD - 1))
                        nc.scalar.activation(qall[:, t, :], pq[:], AF.Identity,
                                             bias=bia[:, t, 0:1])

                # ====================== attention ===============================
                # two-phase per half: all logits+exp first (scalar pipelines,
                # V-gather completes underneath), then the AV matmuls + drains
                for th in range(2):
                    eas = {}
                    for g in range(2):  # gathered half-major: start on first load
                        for t in range(th * 4, th * 4 + 4):
                            for j in range(2):
                                kt = 2 * g + j
                                kc0 = t * TL + j * 128
                                lt0 = psA.tile([128, TL], f32, tag="ps", bufs=4)
                                nc.tensor.matmul(lt0[:], kvg[0:64, g, kc0:kc0 + 128],
                                                 qall[0:64, t, :], start=True, stop=True)
                                lt1 = psA.tile([128, TL], f32, tag="ps", bufs=4)
                                nc.tensor.matmul(lt1[:], kvg[64:128, g, kc0:kc0 + 128],
                                                 qall[64:128, t, :], start=True, stop=True)
                                ea0 = sb2.tile([128, TL], bf16, tag="ea", bufs=32)
                                nc.scalar.activation(ea0[:], lt0[:], AF.Exp,
                                                     bias=msk_sb[:, kt:kt + 1], scale=0.125)
                                ea1 = sb2.tile([128, TL], bf16, tag="ea", bufs=32)
                                nc.scalar.activation(ea1[:], lt1[:], AF.Exp,
                                                     bias=msk_sb[:, kt:kt + 1], scale=0.125)
                                eas[(t, kt)] = (ea0, ea1)
                    for t in range(th * 4, th * 4 + 4):
                        po0 = psA.tile([65, TL], f32, tag="po")
                        po1 = psA.tile([65, TL], f32, tag="po")
                        for kt in range(4):
                            g, j = kt // 2, kt % 2
                            ea0, ea1 = eas[(t, kt)]
                            v0 = KCOLS + j * (H * 65) + (2 * t) * 65
                            nc.tensor.matmul(po0[:], kvg[:, g, v0:v0 + 65], ea0[:],
                                             start=(kt == 0), stop=(kt == 3))
                            nc.tensor.matmul(po1[:], kvg[:, g, v0 + 65:v0 + 130],
                                             ea1[:], start=(kt == 0), stop=(kt == 3))
                        # per-tile softmax normalize, pipelined under tile t+1
                        sums2 = sb2.tile([2, TL], f32, tag="sums2")
                        for pi, po in ((0, po0), (1, po1)):
                            ov = sb2.tile([65, TL], f32, tag="ov")
                            nc.scalar.activation(ov[:], po[:], AF.Copy)
                            nc.scalar.dma_start(oT[pi * 64:pi * 64 + 64, t, :], ov[0:64, :])
                            nc.scalar.dma_start(sums2[pi:pi + 1, :], ov[64:65, :])
                        recip2 = sb2.tile([2, TL], f32r, tag="recip2")
                        with nc.allow_low_precision(reason="softmax recip rounding"):
                            nc.vector.reciprocal(recip2[:], sums2[:])
                        prb = psB.tile([128, TL], f32, tag="aux")
                        nc.tensor.matmul(prb[:], sel2_sb[:], recip2[:],
                                         start=True, stop=True)
                        nc.vector.tensor_tensor(oT[:, t, :], oT[:, t, :], prb[:],
                                                OP.mult)
                        nc.vector.tensor_scalar(oTb[:, t, :], oT[:, t, :],
                                                bia[:, t, 2:3], None, OP.add)
                if debug and l == 0:
                    nc.sync.dma_start(dbg["do"].rearrange("(o p) t -> p o t", p=128), oTb[:])

                # ============ Wo + residual + LN1 stats interleaved =============
                r1 = sb1.tile([128, KD, TL], f32r, tag="r1")
                st1s = psB.tile([1, TL], f32, tag="aux")
                st1q = psB.tile([1, TL], f32, tag="aux")
                for mp in range(4):
                    wc = sb3.tile([128, KD, 256], bf16, tag="wsm", bufs=6)
                    nc.sync.dma_start(wc[:], wrow(Wo[l])[:, :, mp * 256:(mp + 1) * 256])
                    for mi in range(2):
                        m = 2 * mp + mi
                        pa = psA.tile([128, TL], f32, tag="ps", bufs=4)
                        for e in range(KD):
                            nc.tensor.matmul(pa[:], wc[:, e, mi * 128:(mi + 1) * 128],
                                             oTb[:, e, :], start=(e == 0), stop=(e == KD - 1))
                        at = sb2.tile([128, TL], f32, tag="att")
                        nc.scalar.activation(at[:], pa[:], AF.Identity, bias=bia[:, m, 3:4])
                        with nc.allow_low_precision(reason="f32r residual"):
                            nc.vector.tensor_tensor(r1[:, m, :], at[:], hf[:, m, :], OP.add)
                        sq = sb2.tile([128, TL], f32r, tag="sq")
                        with nc.allow_low_precision(reason="f32r squares"):
                            nc.vector.tensor_tensor(sq[:], r1[:, m, :], r1[:, m, :], OP.mult)
                        nc.tensor.matmul(st1s[:], ones_col, r1[:, m, :],
                                         start=(m == 0), stop=(m == KD - 1))
                        nc.tensor.matmul(st1q[:], ones_col, sq[:],
                                         start=(m == 0), stop=(m == KD - 1))
                if debug and l == 0:
                    nc.sync.dma_start(dbg["dr1"].rearrange("(o p) t -> p o t", p=128), r1[:].bitcast(f32))

                h1f = sb1.tile([128, KD, TL], f32, tag="h1f")
                h1b = sb1.tile([128, KD, TL], bf16, tag="h1b")
                ln_finish(st1s, st1q, r1[:], h1b, h1f, bia[:, :, 5:6], bia[:, :, 6:7])
                if debug and l == 0:
                    nc.sync.dma_start(dbg["dh1"].rearrange("(o p) t -> p o t", p=128), h1f[:])

                # ============ FFN with LN2 stats interleaved ====================
                w1v = W1[l].rearrange("(ko kp) m -> kp ko m", kp=128)
                w2v = W2[l].rearrange("(fo fp) m -> fp fo m", fp=128)
                st2s = psB.tile([1, TL], f32, tag="aux")
                st2q = psB.tile([1, TL], f32, tag="aux")
                r2 = y2acc[:].bitcast(f32r)
                for hff in range(4):  # f quarters of 1024
                    uh = sb1.tile([128, 8, TL], bf16, tag="uh")
                    for fp2 in range(2):  # 4 f-tiles per weight strip
                        wc = sb2.tile([128, KD, 512], bf16, tag="wmid", bufs=3)
                        nc.sync.dma_start(
                            wc[:], w1v[:, :, (hff * 2 + fp2) * 512:(hff * 2 + fp2 + 1) * 512])
                        for fi in range(4):
                            fo = fp2 * 4 + fi
                            fg = hff * 8 + fo
                            pu = psA.tile([128, TL], f32, tag="ps", bufs=4)
                            for k in range(KD):
                                nc.tensor.matmul(pu[:], wc[:, k, fi * 128:(fi + 1) * 128],
                                                 h1b[:, k, :],
                                                 start=(k == 0), stop=(k == KD - 1))
                            nc.vector.tensor_scalar(uh[:, fo, :], pu[:],
                                                    b1_sb[:, fg, 0:1], 0.0,
                                                    OP.add, OP.max)
                    if debug and l == 0 and hff < 2:
                        nc.sync.dma_start(
                            dbg["du"][hff * 1024:(hff + 1) * 1024].rearrange(
                                "(o p) t -> p o t", p=128), uh[:])
                    for mp in range(4):
                        wc2 = sb3.tile([128, KD, 256], bf16, tag="wsm", bufs=6)
                        nc.sync.dma_start(wc2[:], w2v[:, hff * 8:(hff + 1) * 8,
                                                      mp * 256:(mp + 1) * 256])
                        for mi in range(2):
                            m = 2 * mp + mi
                            py = psA.tile([128, TL], f32, tag="ps", bufs=4)
                            for fo in range(8):
                                nc.tensor.matmul(py[:], wc2[:, fo, mi * 128:(mi + 1) * 128],
                                                 uh[:, fo, :], start=(fo == 0), stop=(fo == 7))
                            if hff == 0:
                                # y2acc = partial + b2 (bias folded here, added once)
                                with nc.allow_low_precision(reason="f32r partial"):
                                    nc.vector.tensor_scalar(y2acc[:, m, :].bitcast(f32r),
                                                            py[:], bia[:, m, 4:5], None,
                                                            OP.add)
                            elif hff < 3:
                                nc.vector.tensor_tensor(y2acc[:, m, :].bitcast(f32r), py[:],
                                                        y2acc[:, m, :], OP.add)
                            else:
                                tmp = sb2.tile([128, TL], f32, tag="att")
                                nc.vector.tensor_tensor(tmp[:], py[:], y2acc[:, m, :], OP.add)
                                with nc.allow_low_precision(reason="f32r residual"):
                                    nc.vector.tensor_tensor(y2acc[:, m, :].bitcast(f32r),
                                                            tmp[:], h1f[:, m, :], OP.add)
                                sq = sb2.tile([128, TL], f32r, tag="sq")
                                with nc.allow_low_precision(reason="f32r squares"):
                                    nc.vector.tensor_tensor(sq[:], r2[:, m, :], r2[:, m, :],
                                                            OP.mult)
                                nc.tensor.matmul(st2s[:], ones_col, r2[:, m, :],
                                                 start=(m == 0), stop=(m == KD - 1))
                                nc.tensor.matmul(st2q[:], ones_col, sq[:],
                                                 start=(m == 0), stop=(m == KD - 1))
                if debug and l == 0:
                    nc.sync.dma_start(dbg["dr2"].rearrange("(o p) t -> p o t", p=128), y2acc[:])

                ln_finish(st2s, st2q, r2, hb, hf, bia[:, :, 7:8], bia[:, :, 8:9])

            nc.sync.dma_start(out.rearrange("(ko kp) t -> kp ko t", kp=128), hf[:])

    nc.compile()
    return nc


def _sel2():
    sel = np.zeros((2, 128), np.float32)
    for m in range(128):
        sel[m // 64, m] = 1.0
    return sel


def _pos_encoding(position, d):
    pos = np.arange(position)[:, None].astype(np.float32)
    i = np.arange(d)[None, :].astype(np.float32)
    angle = pos / np.power(10000.0, 2.0 * np.floor(i / 2.0) / np.float32(d))
    angle[:, 0::2] = np.sin(angle[:, 0::2])
    angle[:, 1::2] = np.cos(angle[:, 1::2])
    return angle.astype(np.float32)  # [position, d]


def _get_nc():
    if "nc" not in _cache:
        _cache["nc"] = build()
    return _cache["nc"]


def _in_maps(inputs):
    """Host-side prep: full inputs -> per-core input dicts."""
    import ml_dtypes
    inp = {k: np.asarray(v, dtype=np.float32) for k, v in inputs.items()}
    pe = _pos_encoding(MAX_POS, D)[:T]
    x = inp["x"] + pe[None]

    common = {k: inp[k].astype(ml_dtypes.bfloat16)
              for k in ["Wq", "Wk", "Wv", "Wo", "W1", "W2"]}
    pk = lambda a: np.ascontiguousarray(a.reshape(L, KD, 128).transpose(0, 2, 1))
    common["bias9"] = np.ascontiguousarray(np.stack(
        [pk(inp[k]) for k in ["bq", "bk", "bv", "bo", "b2", "g1", "be1", "g2", "be2"]],
        axis=-1))
    common["b1h"] = np.ascontiguousarray(
        inp["b1"].reshape(L, FT, 128).transpose(0, 2, 1)[..., None])
    common["cst"] = np.ones((128, 65), np.float32)
    common["cstb"] = np.ones((128, 32), ml_dtypes.bfloat16)
    common["crow"] = np.ones((65, 128), np.float32)
    common["sel2"] = _sel2()
    in_maps = []
    for c in range(NCORES):
        b, p = c // 2, c % 2
        m = dict(common)
        m["xT"] = np.ascontiguousarray(x[b, p * TL:(p + 1) * TL, :].T)
        mk = (inp["mask"][b, 0, 0] * np.float32(-1e9)).astype(np.float32)
        m["msk"] = np.ascontiguousarray(mk.reshape(4, 128).T)
        in_maps.append(m)
    return in_maps


def kernel(**inputs):
    _, _, _, _, run_bass_kernel_spmd = _imports()
    nc = _get_nc()
    in_maps = _in_maps(inputs)
    res = run_bass_kernel_spmd(nc, in_maps, core_ids=list(range(NCORES)))
    B = np.asarray(inputs["x"]).shape[0]
    out = np.empty((B, T, D), np.float32)
    for b in range(B):
        for p in range(2):
            out[b, p * TL:(p + 1) * TL, :] = res.results[2 * b + p]["out"].T
    return out
